# revision 9
# baseline (speedup 1.0000x reference)
"""Trainium2 Bass kernel for the quirky-reshape MultiHeadAttention module.

Key structural fact: the torch module splits heads with a raw
.view(B, H, T, D) (no transpose), so head h of batch b reads rows
[128h, 128h+128) of (x @ W) and its (T=2048, D=64) q/k/v are just a
reshape of that (128, 1024) slab.  The whole computation therefore
decomposes into B*H = 32 fully independent blocks; each of the 8
NeuronCores handles 4 blocks end-to-end with zero collectives.

Per block (128 input rows):
  - qT/kT projections computed transposed (e' on partitions) so the
    per-head [d=64, t] operand slices fall out as partition windows.
  - time axis processed in a permuted order i=(j, t') with t = 16 t' + j,
    which softmax/attention are equivariant to.
  - scores computed as S^T tiles (k stationary, q moving; row-tiled pairs
    at PE rows 0-63 / 64-127), exp on ACT (no max subtraction needed:
    |S/8| <= ~6), PV with v stationary and an all-ones stationary rider
    at PE column-tile (0,64) producing the softmax row-sums broadcast
    across 64 partitions for free.
"""

import sys

sys.path.insert(0, "/opt/trn_rl_repo")

import numpy as np
import ml_dtypes

B, T, E, H, D = 2, 2048, 1024, 16, 64
NB = 128                 # rows per block
NCORES = 8
BPC = 4                  # blocks per core
CHUNK_J1 = [[0, 2, 4, 6], [8, 10, 12, 14], [1, 3, 5, 7], [9, 11, 13, 15]]

_CACHE = {}


def build_nc():
    import concourse.bass as bass
    import concourse.tile as tile
    from concourse import bacc, mybir

    bf16 = mybir.dt.bfloat16
    f32 = mybir.dt.float32
    Exp = mybir.ActivationFunctionType.Exp

    nc = bacc.Bacc("TRN2", target_bir_lowering=False, debug=False)
    x_in = nc.declare_dram_parameter("x", [BPC * NB, E], bf16, isOutput=False)
    wname = ("wq", "wk", "wv", "wo")
    w_in = {
        n: nc.declare_dram_parameter(n, [E, E], bf16, isOutput=False)
        for n in wname
    }
    out_d = nc.declare_dram_parameter("out", [BPC * NB, E], f32, isOutput=True)

    with tile.TileContext(nc) as tc:
        with (
            tc.tile_pool(name="const", bufs=1) as cpool,
            tc.tile_pool(name="blk", bufs=2) as bpool,
            tc.tile_pool(name="pt", bufs=3) as ptpool,
            tc.tile_pool(name="ps", bufs=2, space="PSUM") as pspool,
        ):
            # ---- persistent per-core tensors ----
            # transposed input: xt[p, g, t'] = x[t', 128 g + p]
            xt = cpool.tile([128, 8, BPC * NB], bf16, tag="xt")
            for g in range(8):
                nc.sync.dma_start_transpose(
                    out=xt[:, g, :], in_=x_in[:, g * 128:(g + 1) * 128]
                )
            # weights: w[p, et, e'] = W[128 et + p, e']
            wsb = {}
            for n in wname:
                wsb[n] = cpool.tile([128, 8, E], bf16, tag=n, name=n + "_sb")
                for et in range(8):
                    nc.gpsimd.dma_start(
                        out=wsb[n][:, et, :],
                        in_=w_in[n][et * 128:(et + 1) * 128, :],
                    )
            ones = cpool.tile([128, 64], bf16, tag="ones")
            nc.vector.memset(ones[:], 1.0)

            for blk in range(BPC):
                tsl = bass.ts(blk, NB)
                # ---------- projections ----------
                # qT/kT transposed: psum[e'-tile, t'] = sum_et WqT . xT
                xqT = bpool.tile([128, 8, NB], bf16, tag="xqT")
                xqTd = bpool.tile([128, 8, NB], bf16, tag="xqTd")
                xkT = bpool.tile([128, 8, NB], bf16, tag="xkT")
                for dst, w in ((xqT, wsb["wq"]), (xkT, wsb["wk"])):
                    for mt in range(8):
                        pq = pspool.tile([128, NB], f32, tag="psp")
                        for et in range(8):
                            nc.tensor.matmul(
                                pq[:],
                                lhsT=w[:, et, bass.ts(mt, 128)],
                                rhs=xt[:, et, tsl],
                                start=(et == 0),
                                stop=(et == 7),
                            )
                        nc.vector.tensor_copy(dst[:, mt, :], pq[:])
                # dup with swapped 64-partition halves
                nc.vector.tensor_copy(xqTd[0:64, :, :], xqT[64:128, :, :])
                nc.vector.tensor_copy(xqTd[64:128, :, :], xqT[0:64, :, :])
                # v natural: psum[t', e'-chunk] = sum_et xT-tile^T . Wv
                xv = bpool.tile([128, E], bf16, tag="xv")
                for ch in range(2):
                    pv = pspool.tile([128, 512], f32, tag="psp")
                    for et in range(8):
                        nc.tensor.matmul(
                            pv[:],
                            lhsT=xt[:, et, tsl],
                            rhs=wsb["wv"][:, et, bass.ts(ch, 512)],
                            start=(et == 0),
                            stop=(et == 7),
                        )
                    nc.vector.tensor_copy(xv[:, bass.ts(ch, 512)], pv[:])

                # ---------- attention ----------
                oslab = bpool.tile([128, 8, NB], bf16, tag="oslab")
                for c in range(4):
                    gb = 0 if c % 2 == 0 else 4
                    nat, dup = (xqT, xqTd) if c < 2 else (xqTd, xqT)
                    rhs0 = nat[0:64, gb:gb + 4, :]      # chunk j1s at base 0
                    rhs64 = dup[64:128, gb:gb + 4, :]   # same j1s at base 64
                    psO = pspool.tile([128, 512], f32, tag="psO")
                    for gp in range(8):
                        pss = pspool.tile([128, 1024], f32, tag="pss")
                        nc.tensor.matmul(
                            pss[:, 0:512], lhsT=xkT[0:64, gp, :], rhs=rhs0,
                            start=True, stop=True,
                        )
                        nc.tensor.matmul(
                            pss[:, 512:1024], lhsT=xkT[64:128, gp, :], rhs=rhs64,
                            start=True, stop=True,
                        )
                        pt = ptpool.tile([128, 1024], bf16, tag="pt")
                        nc.scalar.activation(pt[:], pss[:], Exp, scale=0.125)
                        for half in range(2):
                            j2 = 2 * gp + half
                            first = gp == 0 and half == 0
                            last = gp == 7 and half == 1
                            nc.tensor.matmul(
                                psO[0:64, :],
                                lhsT=xv[:, bass.ts(j2, 64)],
                                rhs=pt[:, bass.ts(half, 512)],
                                start=first, stop=last,
                                skip_group_check=True,
                            )
                            nc.tensor.matmul(
                                psO[64:128, :],
                                lhsT=ones[:],
                                rhs=pt[:, bass.ts(half, 512)],
                                start=first, stop=last,
                                skip_group_check=True,
                            )
                    rinv = ptpool.tile([64, 512], f32, tag="rinv")
                    nc.vector.reciprocal(rinv[:], psO[64:128, :])
                    for s in range(4):
                        j1 = CHUNK_J1[c][s]
                        base = (j1 % 2) * 64
                        nc.vector.tensor_mul(
                            oslab[base:base + 64, j1 // 2, :],
                            psO[0:64, bass.ts(s, 128)],
                            rinv[:, bass.ts(s, 128)],
                        )

                # ---------- output projection ----------
                outf = bpool.tile([128, E], f32, tag="outf")
                for ch in range(2):
                    po = pspool.tile([128, 512], f32, tag="psp")
                    for g in range(8):
                        nc.tensor.matmul(
                            po[:],
                            lhsT=oslab[:, g, :],
                            rhs=wsb["wo"][:, g, bass.ts(ch, 512)],
                            start=(g == 0),
                            stop=(g == 7),
                        )
                    nc.vector.tensor_copy(outf[:, bass.ts(ch, 512)], po[:])
                nc.gpsimd.dma_start(out=out_d[tsl, :], in_=outf[:])

    nc.compile()
    if not nc.is_finalized():
        nc.finalize()
    return nc


# chunk column s -> oslab partition window, must match CHUNK_J1 bookkeeping
def _shard_inputs(x, Wq, Wk, Wv, Wo):
    xb = np.ascontiguousarray(x).astype(ml_dtypes.bfloat16)
    ws = {
        n: np.ascontiguousarray(w).astype(ml_dtypes.bfloat16)
        for n, w in (("wq", Wq), ("wk", Wk), ("wv", Wv), ("wo", Wo))
    }
    in_maps = []
    for core in range(NCORES):
        rows = np.concatenate(
            [
                xb[bi // H, (bi % H) * NB:(bi % H + 1) * NB, :]
                for bi in range(core * BPC, (core + 1) * BPC)
            ],
            axis=0,
        )
        in_maps.append({"x": np.ascontiguousarray(rows), **ws})
    return in_maps


def _unshard(results):
    out = np.zeros((B, T, E), np.float32)
    for core in range(NCORES):
        oc = np.asarray(results[core]["out"], np.float32)
        for j in range(BPC):
            bi = core * BPC + j
            b, h = bi // H, bi % H
            out[b, h * NB:(h + 1) * NB, :] = oc[j * NB:(j + 1) * NB, :]
    return out


def run(x, Wq, Wk, Wv, Wo, trace=False):
    from concourse.bass_utils import run_bass_kernel_spmd

    if "nc" not in _CACHE:
        _CACHE["nc"] = build_nc()
    nc = _CACHE["nc"]
    in_maps = _shard_inputs(x, Wq, Wk, Wv, Wo)
    res = run_bass_kernel_spmd(nc, in_maps, list(range(NCORES)), trace=trace)
    return _unshard(res.results), res


def kernel(x, Wq, Wk, Wv, Wo):
    out, _ = run(x, Wq, Wk, Wv, Wo)
    return out


# revision 11
# speedup vs baseline: 1.2073x; 1.2073x over previous
"""Trainium2 Bass kernel for the quirky-reshape MultiHeadAttention module.

Key structural fact: the torch module splits heads with a raw
.view(B, H, T, D) (no transpose), so head h of batch b reads rows
[128h, 128h+128) of (x @ W) and its (T=2048, D=64) q/k/v are just a
reshape of that (128, 1024) slab.  The whole computation therefore
decomposes into B*H = 32 fully independent blocks; each of the 8
NeuronCores handles 4 blocks end-to-end with zero collectives.

Per block (128 input rows):
  - qT/kT projections computed transposed (e' on partitions) so the
    per-head [d=64, t] operand slices fall out as partition windows.
  - time axis processed in a permuted order i=(j, t') with t = 16 t' + j,
    which softmax/attention are equivariant to.
  - scores computed as S^T tiles (k stationary, q moving; row-tiled pairs
    at PE rows 0-63 / 64-127), exp on ACT (no max subtraction needed:
    |S/8| <= ~6), PV with v stationary and an all-ones stationary rider
    at PE column-tile (0,64) producing the softmax row-sums broadcast
    across 64 partitions for free.
"""

import sys

sys.path.insert(0, "/opt/trn_rl_repo")

import numpy as np
import ml_dtypes

B, T, E, H, D = 2, 2048, 1024, 16, 64
NB = 128                 # rows per block
NCORES = 8
BPC = 4                  # blocks per core
CHUNK_J1 = [[0, 2, 4, 6], [8, 10, 12, 14], [1, 3, 5, 7], [9, 11, 13, 15]]

_CACHE = {}


def build_nc():
    import concourse.bass as bass
    import concourse.tile as tile
    from concourse import bacc, mybir

    bf16 = mybir.dt.bfloat16
    f32 = mybir.dt.float32
    Exp = mybir.ActivationFunctionType.Exp

    nc = bacc.Bacc("TRN2", target_bir_lowering=False, debug=False)
    x_in = nc.declare_dram_parameter("x", [BPC * NB, E], bf16, isOutput=False)
    wname = ("wk", "wq", "wv", "wo")
    w_in = {
        n: nc.declare_dram_parameter(n, [E, E], bf16, isOutput=False)
        for n in wname
    }
    out_d = nc.declare_dram_parameter("out", [BPC * NB, E], f32, isOutput=True)

    with tile.TileContext(nc) as tc:
        with (
            tc.tile_pool(name="const", bufs=1) as cpool,
            tc.tile_pool(name="blk", bufs=2) as bpool,
            tc.tile_pool(name="pt", bufs=3) as ptpool,
            tc.tile_pool(name="ps", bufs=2, space="PSUM") as pspool,
        ):
            # ---- persistent per-core tensors ----
            # transposed input: xt[p, g, t'] = x[t', 128 g + p]
            xt = cpool.tile([128, 8, BPC * NB], bf16, tag="xt")
            for g in range(8):
                nc.sync.dma_start_transpose(
                    out=xt[:, g, :], in_=x_in[:, g * 128:(g + 1) * 128]
                )
            # weights: w[p, et, e'] = W[128 et + p, e']
            wsb = {}
            for n in wname:
                wsb[n] = cpool.tile([128, 8, E], bf16, tag=n, name=n + "_sb")
                for et in range(8):
                    nc.gpsimd.dma_start(
                        out=wsb[n][:, et, :],
                        in_=w_in[n][et * 128:(et + 1) * 128, :],
                    )
            ones = cpool.tile([128, 64], bf16, tag="ones")
            nc.vector.memset(ones[:], 1.0)

            # ---------- projections, all blocks batched (N = 512) ----------
            xqT = cpool.tile([128, 8, BPC * NB], bf16, tag="xqT")
            xqTd = cpool.tile([128, 8, BPC * NB], bf16, tag="xqTd")
            xkT = cpool.tile([128, 8, BPC * NB], bf16, tag="xkT")
            xv = cpool.tile([128, BPC, E], bf16, tag="xv")
            for mt in range(8):
                for dst, w in ((xkT, wsb["wk"]), (xqT, wsb["wq"])):
                    pq = pspool.tile([128, BPC * NB], f32, tag="psp",
                                     name=f"pq_{mt}")
                    for et in range(8):
                        nc.tensor.matmul(
                            pq[:],
                            lhsT=w[:, et, bass.ts(mt, 128)],
                            rhs=xt[:, et, :],
                            start=(et == 0),
                            stop=(et == 7),
                        )
                    nc.vector.tensor_copy(dst[:, mt, :], pq[:])
                # dup with swapped 64-partition halves, per g for fine deps
                nc.vector.tensor_copy(xqTd[0:64, mt, :], xqT[64:128, mt, :])
                nc.vector.tensor_copy(xqTd[64:128, mt, :], xqT[0:64, mt, :])
            # v natural: psum[t', e'-chunk] = sum_et xT-tile^T . Wv
            for blk in range(BPC):
                tsl = bass.ts(blk, NB)
                for ch in range(2):
                    pv = pspool.tile([128, 512], f32, tag="psp",
                                     name=f"pv_{blk}_{ch}")
                    for et in range(8):
                        nc.tensor.matmul(
                            pv[:],
                            lhsT=xt[:, et, tsl],
                            rhs=wsb["wv"][:, et, bass.ts(ch, 512)],
                            start=(et == 0),
                            stop=(et == 7),
                        )
                    nc.vector.tensor_copy(xv[:, blk, bass.ts(ch, 512)], pv[:])

            for blk in range(BPC):
                tsl = bass.ts(blk, NB)
                # ---------- attention ----------
                oslab = bpool.tile([128, 8, NB], bf16, tag="oslab")
                for c in range(4):
                    gb = 0 if c % 2 == 0 else 4
                    nat, dup = (xqT, xqTd) if c < 2 else (xqTd, xqT)
                    rhs0 = nat[0:64, gb:gb + 4, tsl]    # chunk j1s at base 0
                    rhs64 = dup[64:128, gb:gb + 4, tsl]  # same j1s at base 64
                    psO = pspool.tile([128, 512], f32, tag="psO")
                    for gp in range(8):
                        pss = pspool.tile([128, 1024], f32, tag="pss")
                        nc.tensor.matmul(
                            pss[:, 0:512], lhsT=xkT[0:64, gp, tsl], rhs=rhs0,
                            start=True, stop=True,
                        )
                        nc.tensor.matmul(
                            pss[:, 512:1024], lhsT=xkT[64:128, gp, tsl],
                            rhs=rhs64,
                            start=True, stop=True,
                        )
                        pt = ptpool.tile([128, 1024], bf16, tag="pt")
                        nc.scalar.activation(pt[:], pss[:], Exp, scale=0.125)
                        for half in range(2):
                            j2 = 2 * gp + half
                            first = gp == 0 and half == 0
                            last = gp == 7 and half == 1
                            nc.tensor.matmul(
                                psO[0:64, :],
                                lhsT=xv[:, blk, bass.ts(j2, 64)],
                                rhs=pt[:, bass.ts(half, 512)],
                                start=first, stop=last,
                                skip_group_check=True,
                            )
                            nc.tensor.matmul(
                                psO[64:128, :],
                                lhsT=ones[:],
                                rhs=pt[:, bass.ts(half, 512)],
                                start=first, stop=last,
                                skip_group_check=True,
                            )
                    rinv = ptpool.tile([64, 512], f32, tag="rinv")
                    nc.vector.reciprocal(rinv[:], psO[64:128, :])
                    for s in range(4):
                        j1 = CHUNK_J1[c][s]
                        base = (j1 % 2) * 64
                        nc.vector.tensor_mul(
                            oslab[base:base + 64, j1 // 2, :],
                            psO[0:64, bass.ts(s, 128)],
                            rinv[:, bass.ts(s, 128)],
                        )

                # ---------- output projection ----------
                outf = bpool.tile([128, E], f32, tag="outf")
                for ch in range(2):
                    po = pspool.tile([128, 512], f32, tag="psp",
                                     name=f"po_{blk}_{ch}")
                    for g in range(8):
                        nc.tensor.matmul(
                            po[:],
                            lhsT=oslab[:, g, :],
                            rhs=wsb["wo"][:, g, bass.ts(ch, 512)],
                            start=(g == 0),
                            stop=(g == 7),
                        )
                    nc.vector.tensor_copy(outf[:, bass.ts(ch, 512)], po[:])
                nc.gpsimd.dma_start(out=out_d[tsl, :], in_=outf[:])

    nc.compile()
    if not nc.is_finalized():
        nc.finalize()
    return nc


# chunk column s -> oslab partition window, must match CHUNK_J1 bookkeeping
def _shard_inputs(x, Wq, Wk, Wv, Wo):
    xb = np.ascontiguousarray(x).astype(ml_dtypes.bfloat16)
    ws = {
        n: np.ascontiguousarray(w).astype(ml_dtypes.bfloat16)
        for n, w in (("wq", Wq), ("wk", Wk), ("wv", Wv), ("wo", Wo))
    }
    in_maps = []
    for core in range(NCORES):
        rows = np.concatenate(
            [
                xb[bi // H, (bi % H) * NB:(bi % H + 1) * NB, :]
                for bi in range(core * BPC, (core + 1) * BPC)
            ],
            axis=0,
        )
        in_maps.append({"x": np.ascontiguousarray(rows), **ws})
    return in_maps


def _unshard(results):
    out = np.zeros((B, T, E), np.float32)
    for core in range(NCORES):
        oc = np.asarray(results[core]["out"], np.float32)
        for j in range(BPC):
            bi = core * BPC + j
            b, h = bi // H, bi % H
            out[b, h * NB:(h + 1) * NB, :] = oc[j * NB:(j + 1) * NB, :]
    return out


def run(x, Wq, Wk, Wv, Wo, trace=False):
    from concourse.bass_utils import run_bass_kernel_spmd

    if "nc" not in _CACHE:
        _CACHE["nc"] = build_nc()
    nc = _CACHE["nc"]
    in_maps = _shard_inputs(x, Wq, Wk, Wv, Wo)
    res = run_bass_kernel_spmd(nc, in_maps, list(range(NCORES)), trace=trace)
    return _unshard(res.results), res


def kernel(x, Wq, Wk, Wv, Wo):
    out, _ = run(x, Wq, Wk, Wv, Wo)
    return out
